# revision 36
# baseline (speedup 1.0000x reference)
"""Sparse BertSelfAttention TRN2 kernel (8 NeuronCores, SPMD).

Sharding: core c -> (batch b = c//2, head-half = c%2).  Each core computes the
full attention for 6 of the 12 heads of one batch: output channels
[half*384, half*384+384) of out[b].

Host (shard step) gathers the pruned token rows and pre-transposes them
(pure data movement): xqT = hidden[b][q_idx].T, xkvT = hidden[b][kv_idx].T,
both bf16 [768, 1024] laid out as [6, 128, 1024].  Weights are passed as
transposed bf16 slices [6, 128, 384].

Device (all FLOPs, bf16 matmuls accumulating in fp32 PSUM):
  qgT = Wq_sl.T @ xqT + bq     [384, 1024]   (ch-partition layout)
  kgT likewise from xkvT; vg = xkvT.T @ Wv_sl + bv  [kv, 384] (+ ones col
  per head -> vga [kv, 6*65])
  per head h: S^T[j,i] = kg_h @ qg_h^T ; expS = exp(S^T/8 + mask_j)
              pv[0:64] = vg_h.T @ expS (unnormalized ctx^T), pv[64] = rowsum
  pm = sum_j exp(mask_j) * vga_j   (weighted-mean numerators + denominator)
Outputs: ctxout [6*65, 1024] fp32 (raw pv), pmout [390] fp32.

Host (unshard step): ctx = pv[:64]/pv[64] transposed to [1024, 384], written
at rows q_idx; rows not in q_idx get vmean = pm[:64*6]/pm[64::65] (softmax of
an all-masked row is the exp(mask)-weighted mean of v).
"""
import threading

import numpy as np

B, T, H = 4, 2048, 768
NH, DH = 12, 64
KQ, KKV = 1024, 1024
O = 384          # output channels per core
NHC = 6          # heads per core
N_CORES = 8

_lock = threading.Lock()
_state = {}


def _build(repeat=1):
    import concourse.bass as bass
    import concourse.bacc as bacc
    import concourse.tile as tile
    from concourse import mybir

    P = 128
    f32 = mybir.dt.float32
    bf16 = mybir.dt.bfloat16
    EXP = mybir.ActivationFunctionType.Exp

    nc = bacc.Bacc(None, target_bir_lowering=False, debug=False,
                   num_swdge_queues=1)

    # DRAM tensors (shapes as the host ships them)
    xqt = nc.dram_tensor("xqt", [NHC, P, KQ], bf16, kind="ExternalInput")
    xkvt = nc.dram_tensor("xkvt", [NHC, P, KKV], bf16, kind="ExternalInput")
    wqt = nc.dram_tensor("wqt", [NHC, P, O], bf16, kind="ExternalInput")
    wkt = nc.dram_tensor("wkt", [NHC, P, O], bf16, kind="ExternalInput")
    wvt = nc.dram_tensor("wvt", [NHC, P, O], bf16, kind="ExternalInput")
    bq = nc.dram_tensor("bq", [O], f32, kind="ExternalInput")
    bk = nc.dram_tensor("bk", [O], f32, kind="ExternalInput")
    bvb = nc.dram_tensor("bvb", [P, O], f32, kind="ExternalInput")
    maskkv = nc.dram_tensor("maskkv", [KKV], f32, kind="ExternalInput")
    expmask = nc.dram_tensor("expmask", [KKV], f32, kind="ExternalInput")
    ctxout = nc.dram_tensor("ctxout", [NHC * 65, KQ], bf16,
                            kind="ExternalOutput")
    pmout = nc.dram_tensor("pmout", [1, NHC * 65], f32, kind="ExternalOutput")

    NHB = H // P           # 6 hidden-dim tiles (kh)
    NMO = O // P           # 3 output-channel tiles (mo)
    NJT = KKV // P         # 8 kv-row tiles (mj)
    NNI = 2                # 512-wide q column halves (ni)
    VW = NHC * 65          # 390: vga row width per kv tile

    with tile.TileContext(nc) as tc:
      for rep in range(repeat):
        sfx = f"_{rep}"
        with (
            tc.tile_pool(name="const" + sfx, bufs=1) as const,
            tc.tile_pool(name="perm" + sfx, bufs=1) as perm,
            tc.tile_pool(name="ps_s" + sfx, bufs=1, space="PSUM") as ps_s,
            tc.tile_pool(name="ep" + sfx, bufs=1) as ep,
        ):
            # ---------- tiles for small constants ----------
            maskkv_sb = const.tile([P, NJT], f32, name="maskkv_sb")
            expmask_sb = const.tile([P, NJT], f32, name="expmask_sb")
            bq_sb = const.tile([P, NMO], f32, name="bq_sb")
            bk_sb = const.tile([P, NMO], f32, name="bk_sb")
            bvb_sb = const.tile([P, O], f32, name="bvb_sb")
            ones6 = const.tile([P, NHC], bf16, name="ones6")
            nc.vector.memset(ones6[:], 1.0)
            # zero tiles for PE-warmup matmuls (pstate ramp-up during DMA)
            zq = const.tile([P, P], bf16, name="zq")
            nc.vector.memset(zq[:], 0.0)
            zx = const.tile([P, 64], bf16, name="zx")
            nc.vector.memset(zx[:], 0.0)

            # ---------- persistent tiles ----------
            xq_sb = perm.tile([P, NHB * KQ], bf16, name="xq_sb")
            xkv_sb = perm.tile([P, NHB * KKV], bf16, name="xkv_sb")
            wq_sb = perm.tile([P, NHB * O], bf16, name="wq_sb")
            wk_sb = perm.tile([P, NHB * O], bf16, name="wk_sb")
            wv_sb = perm.tile([P, NHB * O], bf16, name="wv_sb")
            qgT = perm.tile([P, NMO * KQ], bf16, name="qgT")
            kgT = perm.tile([P, NMO * KKV], bf16, name="kgT")
            vga = perm.tile([P, NJT * VW], bf16, name="vga")

            # ---------- big input DMAs (order = DMA_ENGINES order) ----------
            def load_w(w_sb, wt):
                nc.sync.dma_start(
                    out=bass.AP(w_sb.tensor, w_sb[:].offset,
                                [w_sb[:].ap[0], [O, NHB], [1, O]]),
                    in_=bass.AP(wt, 0, [[O, P], [P * O, NHB], [1, O]]),
                )

            def load_x(x_sb, xt, half, width, kh0=0, kh1=NHB):
                # token-half DMA: kh blocks [kh0, kh1), tokens [half*512, +512)
                nc.sync.dma_start(
                    out=bass.AP(x_sb.tensor,
                                x_sb[:].offset + kh0 * width + half * 512,
                                [x_sb[:].ap[0], [width, kh1 - kh0], [1, 512]]),
                    in_=bass.AP(xt, kh0 * P * width + half * 512,
                                [[width, P], [P * width, kh1 - kh0], [1, 512]]),
                )

            # order = DMA_ENGINES service order, matched to PE emission order
            # (q-mo01-ni0, k-mo01-ni0/ni1, q-mo0-ni1, scores h0...) so the PE
            # stream never stalls once started.
            load_w(wq_sb, wqt)
            load_x(xq_sb, xqt, 0, KQ, 0, 3)
            load_x(xq_sb, xqt, 0, KQ, 3, NHB)
            load_w(wk_sb, wkt)
            nc.sync.dma_start(out=bq_sb[:],
                              in_=bass.AP(bq, 0, [[1, P], [P, NMO]]))
            nc.sync.dma_start(out=bk_sb[:],
                              in_=bass.AP(bk, 0, [[1, P], [P, NMO]]))
            load_x(xkv_sb, xkvt, 0, KKV, 0, 3)
            load_x(xkv_sb, xkvt, 0, KKV, 3, NHB)
            nc.sync.dma_start(out=maskkv_sb[:],
                              in_=bass.AP(maskkv, 0, [[1, P], [P, NJT]]))
            nc.sync.dma_start(out=expmask_sb[:],
                              in_=bass.AP(expmask, 0, [[1, P], [P, NJT]]))
            load_x(xkv_sb, xkvt, 1, KKV)
            load_x(xq_sb, xqt, 1, KQ)
            load_w(wv_sb, wvt)
            nc.sync.dma_start(out=bvb_sb[:],
                              in_=bass.AP(bvb, 0, [[O, P], [1, O]]))

            # ---------- emission helpers ----------
            def emit_qk_group(w_sb, b_sb, gT, x_sb, width, mo, ni, psp):
                pj = psp.tile([P, 512], f32, tag="pj", bufs=3,
                              name=f"pj{id(w_sb) % 97}_{mo}_{ni}{sfx}")
                for kh in range(NHB):
                    nc.tensor.matmul(
                        pj[:],
                        w_sb[:, kh * O + mo * P: kh * O + (mo + 1) * P],
                        x_sb[:, kh * width + ni * 512: kh * width + ni * 512 + 512],
                        start=(kh == 0), stop=(kh == NHB - 1),
                    )
                nc.vector.tensor_scalar_add(
                    gT[:, mo * width + ni * 512: mo * width + ni * 512 + 512],
                    pj[:], b_sb[:, mo:mo + 1],
                )

            def emit_qk_groups_split(groups, psp):
                """Emit kh 0-2 of every group, then kh 3-5, then drains --
                lets the first groups start before the second x-chunk DMA."""
                tiles = []
                for w_sb, b_sb, gT, x_sb, width, mo, ni in groups:
                    pj = psp.tile([P, 512], f32, tag="pj", bufs=3,
                                  name=f"pjs{id(w_sb) % 97}_{mo}_{ni}{sfx}")
                    tiles.append(pj)
                    for kh in range(3):
                        nc.tensor.matmul(
                            pj[:],
                            w_sb[:, kh * O + mo * P: kh * O + (mo + 1) * P],
                            x_sb[:, kh * width + ni * 512:
                                 kh * width + ni * 512 + 512],
                            start=(kh == 0), stop=False,
                        )
                for pj, (w_sb, b_sb, gT, x_sb, width, mo, ni) in zip(
                        tiles, groups):
                    for kh in range(3, NHB):
                        nc.tensor.matmul(
                            pj[:],
                            w_sb[:, kh * O + mo * P: kh * O + (mo + 1) * P],
                            x_sb[:, kh * width + ni * 512:
                                 kh * width + ni * 512 + 512],
                            start=False, stop=(kh == NHB - 1),
                        )
                for pj, (w_sb, b_sb, gT, x_sb, width, mo, ni) in zip(
                        tiles, groups):
                    nc.vector.tensor_scalar_add(
                        gT[:, mo * width + ni * 512: mo * width + ni * 512 + 512],
                        pj[:], b_sb[:, mo:mo + 1],
                    )

            def emit_v_group(mj, psp):
                pj = psp.tile([P, 512], f32, tag="pj", bufs=3,
                              name=f"pjv_{mj}{sfx}")
                for kh in range(NHB):
                    nc.tensor.matmul(
                        pj[:, 0:O],
                        xkv_sb[:, kh * KKV + mj * P: kh * KKV + (mj + 1) * P],
                        wv_sb[:, kh * O:(kh + 1) * O],
                        start=(kh == 0), stop=(kh == NHB - 1),
                    )
                base = vga[:].offset + mj * VW
                nc.vector.tensor_copy(
                    out=bass.AP(vga.tensor, base + 64,
                                [vga[:].ap[0], [65, NHC], [1, 1]]),
                    in_=bass.AP(ones6.tensor, ones6[:].offset,
                                [ones6[:].ap[0], [1, NHC], [1, 1]]),
                )
                nc.vector.tensor_tensor(
                    out=bass.AP(vga.tensor, base,
                                [vga[:].ap[0], [65, NHC], [1, DH]]),
                    in0=bass.AP(pj.tensor, pj[:].offset,
                                [pj[:].ap[0], [DH, NHC], [1, DH]]),
                    in1=bass.AP(bvb_sb.tensor, bvb_sb[:].offset,
                                [bvb_sb[:].ap[0], [DH, NHC], [1, DH]]),
                    op=mybir.AluOpType.add,
                )

            expS = {}

            def s_matmul(s_ps, h, mj, ni):
                hp = (h % 2) * DH
                mo = h // 2
                nc.tensor.matmul(
                    s_ps[:, ni * 512:(ni + 1) * 512],
                    kgT[hp:hp + DH,
                        mo * KKV + mj * P: mo * KKV + (mj + 1) * P],
                    qgT[hp:hp + DH,
                        mo * KQ + ni * 512: mo * KQ + (ni + 1) * 512],
                    start=True, stop=True,
                )

            def emit_scores(h, split01=False):
                """S^T + exp for one head; expS[h] [128, 8*1024] bf16.

                split01: emit mj0/mj1 as ni-interleaved units so their ni0
                matmuls run before qgT-ni1 exists (earlier act start)."""
                eS = ep.tile([P, NJT * KQ], bf16, tag="expS", bufs=5,
                             name=f"expS{h}{sfx}")
                expS[h] = eS
                s_tiles = {}
                first = 0
                if split01:
                    first = 2
                    for mj in range(2):
                        s_tiles[mj] = ps_s.tile([P, KQ], f32, tag="s", bufs=2,
                                                name=f"s{h}_{mj}{sfx}")
                        s_matmul(s_tiles[mj], h, mj, 0)
                    for mj in range(2):
                        s_matmul(s_tiles[mj], h, mj, 1)
                        nc.scalar.activation(
                            eS[:, mj * KQ:(mj + 1) * KQ], s_tiles[mj][:], EXP,
                            bias=maskkv_sb[:, mj:mj + 1], scale=0.125)
                for mj in range(first, NJT):
                    s_ps = ps_s.tile([P, KQ], f32, tag="s", bufs=2,
                                     name=f"s{h}_{mj}{sfx}")
                    for ni in range(NNI):
                        s_matmul(s_ps, h, mj, ni)
                    nc.scalar.activation(
                        eS[:, mj * KQ:(mj + 1) * KQ], s_ps[:], EXP,
                        bias=maskkv_sb[:, mj:mj + 1], scale=0.125)

            def emit_pv(h, pvp, cp, last=False):
                eS = expS[h]
                for ni in range(NNI):
                    pvt = pvp.tile([65, 512], f32, tag=f"pv{ni}", bufs=2,
                                   name=f"pv{ni}_{h}{sfx}")
                    for mj in range(NJT):
                        nc.tensor.matmul(
                            pvt[:],
                            vga[:, mj * VW + h * 65: mj * VW + h * 65 + 65],
                            eS[:, mj * KQ + ni * 512: mj * KQ + ni * 512 + 512],
                            start=(mj == 0), stop=(mj == NJT - 1),
                        )
                    ctx_sb = cp.tile([65, 512], bf16, tag="ctx", bufs=3,
                                     name=f"ctx{h}_{ni}{sfx}")
                    nchunk = 1
                    cw = 512
                    for ch in range(nchunk):
                        nc.vector.tensor_copy(
                            out=ctx_sb[:, ch * cw:(ch + 1) * cw],
                            in_=pvt[:, ch * cw:(ch + 1) * cw])
                        nc.sync.dma_start(
                            out=bass.AP(ctxout,
                                        (h * 65) * KQ + ni * 512 + ch * cw,
                                        [[KQ, 65], [1, cw]]),
                            in_=ctx_sb[:, ch * cw:(ch + 1) * cw],
                        )

            # ---------- phase 1: projections + scores (PSUM: s=4, pj=3, pm=1)
            with tc.tile_pool(name="psp" + sfx, bufs=1, space="PSUM") as psp:
                # PE warmup: dummy matmuls ramp the tensor engine to full
                # p-state while the first DMAs are still in flight.  The
                # p-state is sampled at instruction dispatch (~18 instrs ahead
                # of execution), so use many small matmuls spanning >3us.
                warm = psp.tile([P, 512], f32, tag="pj", bufs=3,
                                name="warm" + sfx)
                for _ in range(100):
                    nc.tensor.matmul(warm[:, 0:64], zq[:], zx[:],
                                     start=True, stop=True)
                emit_qk_groups_split(
                    [(wq_sb, bq_sb, qgT, xq_sb, KQ, 0, 0),
                     (wq_sb, bq_sb, qgT, xq_sb, KQ, 1, 0),
                     (wq_sb, bq_sb, qgT, xq_sb, KQ, 2, 0)], psp)
                emit_qk_groups_split(
                    [(wk_sb, bk_sb, kgT, xkv_sb, KKV, 0, 0),
                     (wk_sb, bk_sb, kgT, xkv_sb, KKV, 1, 0),
                     (wk_sb, bk_sb, kgT, xkv_sb, KKV, 2, 0)], psp)
                emit_qk_group(wk_sb, bk_sb, kgT, xkv_sb, KKV, 0, 1, psp)
                emit_qk_group(wk_sb, bk_sb, kgT, xkv_sb, KKV, 1, 1, psp)
                emit_qk_group(wq_sb, bq_sb, qgT, xq_sb, KQ, 0, 1, psp)
                emit_scores(0, split01=True)
                emit_scores(1)
                emit_v_group(0, psp)
                emit_v_group(1, psp)
                emit_v_group(2, psp)
                emit_v_group(3, psp)
                emit_qk_group(wq_sb, bq_sb, qgT, xq_sb, KQ, 1, 1, psp)
                emit_scores(2)
                emit_scores(3)
                for mj in range(4, NJT):
                    emit_v_group(mj, psp)
                emit_qk_group(wq_sb, bq_sb, qgT, xq_sb, KQ, 2, 1, psp)
                emit_qk_group(wk_sb, bk_sb, kgT, xkv_sb, KKV, 2, 1, psp)
                # weighted-mean numerators/denominators over all kv rows:
                # DVE multiply-accumulate, then one ones-vector matmul to sum
                # over partitions (163ns on PE vs 1.3us for 8 direct matmuls)
                f32r = mybir.dt.float32r
                wm = perm.tile([P, VW], f32r, name="wm" + sfx)
                wt = perm.tile([P, VW], f32r, name="wt" + sfx)
                ones1f = const.tile([P, 1], f32, name="ones1f" + sfx)
                nc.vector.memset(ones1f[:], 1.0)
                ones1r = const.tile([P, 1], f32r, name="ones1r" + sfx)
                nc.vector.tensor_copy(out=ones1r[:], in_=ones1f[:])
                nc.vector.tensor_scalar_mul(
                    wm[:], vga[:, 0:VW], expmask_sb[:, 0:1])
                for mj in range(1, NJT):
                    nc.vector.tensor_scalar_mul(
                        wt[:], vga[:, mj * VW:(mj + 1) * VW],
                        expmask_sb[:, mj:mj + 1])
                    nc.vector.tensor_tensor(
                        out=wm[:], in0=wm[:], in1=wt[:],
                        op=mybir.AluOpType.add)
                pm_ps = psp.tile([1, 512], f32, tag="pm", bufs=1,
                                 name="pm_ps" + sfx)
                nc.tensor.matmul(pm_ps[:1, 0:VW], ones1r[:], wm[:],
                                 start=True, stop=True)
                pm_sb = perm.tile([1, VW], f32, name="pm_sb" + sfx)
                nc.vector.tensor_copy(out=pm_sb[:], in_=pm_ps[:1, 0:VW])
                nc.sync.dma_start(
                    out=bass.AP(pmout, 0, [[VW, 1], [1, VW]]),
                    in_=pm_sb[:],
                )

            # ---------- phase 2: tail scores + PV (PSUM: s=4 + pv=4) ------
            with tc.tile_pool(name="pvp" + sfx, bufs=1, space="PSUM") as pvp, \
                 tc.tile_pool(name="cp" + sfx, bufs=1) as cp:
                emit_scores(4)
                emit_pv(0, pvp, cp)
                emit_scores(5)
                for h in range(1, NHC):
                    emit_pv(h, pvp, cp, last=(h == NHC - 1))

    nc.compile()
    return nc


def _get_runner():
    """Build (once) a reusable jitted SPMD callable over 8 cores."""
    with _lock:
        if "runner" in _state:
            return _state["runner"]

        import jax
        from jax.sharding import Mesh, PartitionSpec
        from jax.experimental.shard_map import shard_map
        from concourse import mybir
        from concourse import bass2jax

        nc = _build()
        bass2jax.install_neuronx_cc_hook()

        partition_name = (
            nc.partition_id_tensor.name if nc.partition_id_tensor else None
        )
        in_names, out_names, out_avals, zero_outs = [], [], [], []
        for alloc in nc.m.functions[0].allocations:
            if not isinstance(alloc, mybir.MemoryLocationSet):
                continue
            name = alloc.memorylocations[0].name
            if alloc.kind == "ExternalInput":
                if name != partition_name:
                    in_names.append(name)
            elif alloc.kind == "ExternalOutput":
                out_names.append(name)
                shape = tuple(alloc.tensor_shape)
                dtype = mybir.dt.np(alloc.dtype)
                out_avals.append(jax.core.ShapedArray(shape, dtype))
                zero_outs.append(np.zeros(shape, dtype))
        n_params = len(in_names)
        all_names = in_names + out_names
        if partition_name is not None:
            all_names = all_names + [partition_name]

        def _body(*args):
            operands = list(args)
            if partition_name is not None:
                operands.append(bass2jax.partition_id_tensor())
            outs = bass2jax._bass_exec_p.bind(
                *operands,
                out_avals=tuple(out_avals),
                in_names=tuple(all_names),
                out_names=tuple(out_names),
                lowering_input_output_aliases=(),
                sim_require_finite=True,
                sim_require_nnan=True,
                nc=nc,
            )
            return tuple(outs)

        try:
            devices = jax.devices("axon")[:N_CORES]
        except RuntimeError:
            devices = jax.devices()[:N_CORES]
        mesh = Mesh(np.asarray(devices), ("core",))
        n_out = len(out_names)
        sharded = jax.jit(
            shard_map(
                _body, mesh=mesh,
                in_specs=(PartitionSpec("core"),) * (n_params + n_out),
                out_specs=(PartitionSpec("core"),) * n_out,
                check_rep=False,
            ),
            donate_argnums=tuple(range(n_params, n_params + n_out)),
            keep_unused=True,
        )

        def run(in_maps):
            concat_in = [
                np.concatenate([np.asarray(in_maps[c][nm]) for c in range(N_CORES)],
                               axis=0)
                for nm in in_names
            ]
            concat_zero = [
                np.concatenate([z for _ in range(N_CORES)], axis=0) for z in zero_outs
            ]
            out_arrs = sharded(*concat_in, *concat_zero)
            out_arrs = [np.asarray(a) for a in out_arrs]
            results = []
            for c in range(N_CORES):
                m = {}
                for i, nm in enumerate(out_names):
                    sh0 = out_avals[i].shape[0]
                    m[nm] = out_arrs[i][c * sh0:(c + 1) * sh0]
                results.append(m)
            return results

        _state["runner"] = run
        return run


def _shard_inputs(hidden_states, attention_mask, Wq, bq, Wk, bk, Wv, bv,
                  q_indices, kv_indices):
    import ml_dtypes
    bf16 = ml_dtypes.bfloat16
    in_maps = []
    hidden_states = np.asarray(hidden_states, dtype=np.float32)
    attention_mask = np.asarray(attention_mask, dtype=np.float32)
    for c in range(N_CORES):
        b, half = c // 2, c % 2
        o0 = half * O
        qi = np.asarray(q_indices[b], dtype=np.int64)
        kvi = np.asarray(kv_indices[b], dtype=np.int64)
        xq = hidden_states[b][qi]                      # [1024, 768]
        xkv = hidden_states[b][kvi]
        mkv = np.ascontiguousarray(attention_mask[b, 0, 0, kvi])
        in_maps.append({
            "xqt": np.ascontiguousarray(xq.T).astype(bf16).reshape(NHC, 128, KQ),
            "xkvt": np.ascontiguousarray(xkv.T).astype(bf16).reshape(NHC, 128, KKV),
            "wqt": np.ascontiguousarray(Wq[o0:o0 + O, :].T).astype(bf16).reshape(NHC, 128, O),
            "wkt": np.ascontiguousarray(Wk[o0:o0 + O, :].T).astype(bf16).reshape(NHC, 128, O),
            "wvt": np.ascontiguousarray(Wv[o0:o0 + O, :].T).astype(bf16).reshape(NHC, 128, O),
            "bq": np.ascontiguousarray(bq[o0:o0 + O], dtype=np.float32),
            "bk": np.ascontiguousarray(bk[o0:o0 + O], dtype=np.float32),
            "bvb": np.ascontiguousarray(
                np.broadcast_to(bv[o0:o0 + O], (128, O)), dtype=np.float32),
            "maskkv": mkv,
            "expmask": np.exp(mkv),
        })
    return in_maps


def kernel(hidden_states, attention_mask, Wq, bq, Wk, bk, Wv, bv,
           q_indices, kv_indices):
    run = _get_runner()
    in_maps = _shard_inputs(hidden_states, attention_mask, Wq, bq, Wk, bk, Wv, bv,
                            q_indices, kv_indices)
    results = run(in_maps)
    out = np.empty((B, T, NH * DH), dtype=np.float32)
    for c in range(N_CORES):
        b, half = c // 2, c % 2
        o0 = half * O
        qi = np.asarray(q_indices[c // 2], dtype=np.int64)
        ctxT = np.asarray(results[c]["ctxout"], dtype=np.float32).reshape(
            NHC, 65, KQ)
        denom = ctxT[:, 64:65, :]
        ctx = ctxT[:, :64, :] / np.where(denom == 0.0, 1.0, denom)
        pm = results[c]["pmout"].reshape(NHC, 65)
        pden = pm[:, 64:65]
        vmean = (pm[:, :64] / np.where(pden == 0.0, 1.0, pden)).reshape(O)
        nmask = np.ones(T, dtype=bool)
        nmask[qi] = False
        out[b, nmask, o0:o0 + O] = vmean[None, :]
        out[b, qi, o0:o0 + O] = ctx.transpose(2, 0, 1).reshape(KQ, O)
    return out
